# revision 11
# baseline (speedup 1.0000x reference)
"""CrossDepthAttention Trainium2 kernel.

Computation (per token t):
    q = x_t @ Wq.T
    k_n = h_{t,n} @ Wk.T, v_n = h_{t,n} @ Wv.T   for n in 0..7
    logits[h,n] = <q_h, k_{n,h}> / sqrt(Hd)
    attn = softmax_n(logits)
    out_h = sum_n attn[h,n] * v_{n,h}
    y_t = x_t + out @ Wo.T

Sharding: pure data-parallel over the B*S = 8192 tokens, 1024 tokens per
core on 8 cores (every token attends only to its own history: no
cross-token coupling, no collective).

Per-core kernel, token tiles of 128.  All 18 projections run on the
tensor engine in fp8e4m3 with the DoubleRow perf mode: stationary
[128, 2, 128] / moving [128, 2, 512] pairs contract 256 deep per output
column.  Measured 1.03 cyc/out-col with the stationary reused across the
two 512-wide psum halves (dc outer, j inner) -- a 2.05x throughput win
over bf16; the doubled ldweights hides completely only when the
stationary is reused by back-to-back matmuls.  End-to-end rel err with
everything fp8 is 1.2e-2 vs the 2e-2 budget (numpy-emulated and
HW-confirmed; fp32 psum accumulation throughout).

Attention runs on DVE/ACT in bf16 with every big TensorTensor shaped for
the DVE 2x_1p mode (all operands 2-byte, innermost stride 1):
  * k and v drain PSUM->SBUF bf16 on the ACT engine (so PSUM recycles at
    matmul pace and the q*k product reads 2-byte SBUF, not 4-byte PSUM).
  * v is stored [t, d, n] so the attn broadcast multiply and the
    n-reduction tree are innermost-contiguous (stride-1 over n).
  * logits = per-head sum of q*k: two 2x tree levels then one short
    strided reduce (TensorReduce has no fast mode, so shrink its input).
  * the fp32 residual add runs on the otherwise-idle Pool engine.
"""

import numpy as np
import ml_dtypes
from contextlib import ExitStack

import concourse.bass as bass
import concourse.mybir as mybir
import concourse.tile as tile
from concourse import bacc
from concourse.masks import make_identity

BF16 = mybir.dt.bfloat16
FP8 = mybir.dt.float8e4
F32 = mybir.dt.float32

B, S, D = 4, 2048, 1024
NPREV = 8
H = 16
HD = D // H  # 64
BS = B * S
N_CORES = 8
T_CORE = BS // N_CORES  # 1024 tokens per core
P = 128  # partition / token-tile size
C = D // P  # 8 contraction chunks; DC = 4 DoubleRow double-chunks
DC = C // 2
N_TILES = T_CORE // P  # 8 token tiles per core
HALF = 512  # matmul free-dim (one PSUM bank)
SCALE = 1.0 / float(np.sqrt(HD))

_CACHE = {}


UNROLL = 8


def build_program(n_tiles=N_TILES, repeat=1, mm_only=False):
    """Build the single-core Bass/Tile program (run SPMD on 8 cores).

    repeat>1 wraps the whole computation in an on-device For_i loop of
    repeat//UNROLL iterations with UNROLL copies of the body each --
    used only for timing (amortizes host dispatch overhead away, and the
    partial unroll amortizes For_i's per-iteration all-engine barrier,
    letting consecutive kernel executions overlap through pool WAR deps).
    """
    nc = bacc.Bacc("TRN2", debug=False, num_devices=N_CORES)
    t_tok = n_tiles * P

    curP = nc.dram_tensor("curP", [n_tiles, P, DC, 2, P], FP8, kind="ExternalInput").ap()
    histP = nc.dram_tensor(
        "histP", [n_tiles, NPREV, P, DC, 2, P], FP8, kind="ExternalInput"
    ).ap()
    xres = nc.dram_tensor("xres", [t_tok, D], F32, kind="ExternalInput").ap()
    wq = nc.dram_tensor("wqT", [DC, P, 2, D], FP8, kind="ExternalInput").ap()
    wk = nc.dram_tensor("wkT", [DC, P, 2, D], FP8, kind="ExternalInput").ap()
    wv = nc.dram_tensor("wvT", [DC, P, 2, D], FP8, kind="ExternalInput").ap()
    wo = nc.dram_tensor("woT", [DC, P, 2, D], FP8, kind="ExternalInput").ap()
    y = nc.dram_tensor("y", [t_tok, D], F32, kind="ExternalOutput").ap()

    with ExitStack() as ctx:
        ctx.enter_context(
            nc.allow_low_precision(reason="fp8/bf16 attn validated vs 2e-2")
        )
        tc = ctx.enter_context(tile.TileContext(nc))

        wpool = ctx.enter_context(tc.tile_pool(name="wpool", bufs=1))
        inpool = ctx.enter_context(tc.tile_pool(name="inpool", bufs=2))
        xrpool = ctx.enter_context(tc.tile_pool(name="xrpool", bufs=3))
        qpool = ctx.enter_context(tc.tile_pool(name="qpool", bufs=2))
        kpool = ctx.enter_context(tc.tile_pool(name="kpool", bufs=2))
        vpool = ctx.enter_context(tc.tile_pool(name="vpool", bufs=2))
        ppool = ctx.enter_context(tc.tile_pool(name="ppool", bufs=2))
        spool = ctx.enter_context(tc.tile_pool(name="spool", bufs=2))
        aopool = ctx.enter_context(tc.tile_pool(name="aopool", bufs=2))
        atpool = ctx.enter_context(tc.tile_pool(name="atpool", bufs=2))
        ypool = ctx.enter_context(tc.tile_pool(name="ypool", bufs=2))
        psum = ctx.enter_context(tc.tile_pool(name="psum", bufs=4, space="PSUM"))

        # Resident weights, DoubleRow moving layout:
        # w8_sb[p, dc, two, dout] = W?T[(2dc+two)*128+p, dout]
        w_sb = {}
        for name, dram in (("wq", wq), ("wk", wk), ("wv", wv), ("wo", wo)):
            wt = wpool.tile([P, DC, 2, D], FP8, name=f"{name}_sb", tag=name)
            nc.sync.dma_start(wt[:], dram.rearrange("dc p two d -> p dc two d"))
            w_sb[name] = wt
        ident = wpool.tile([P, P], BF16, name="ident", tag="ident")
        make_identity(nc, ident[:])

        def proj_dr(out_psum, stat, w_name, accumulate=False):
            """fp8 DoubleRow projection: out[t,:] (+)= stat.T @ W.T with
            256-deep contraction per column.  dc outer / j inner so each
            stationary serves two back-to-back matmuls -- that reuse is
            what hides the doubled ldweights (measured 1.03 cyc/col vs
            1.26 with j outer)."""
            wt = w_sb[w_name]
            for dc in range(DC):
                for j in range(2):
                    nc.tensor.matmul(
                        out_psum[:, j * HALF : (j + 1) * HALF],
                        lhsT=stat(dc),
                        rhs=wt[:, dc, :, j * HALF : (j + 1) * HALF],
                        start=(dc == 0 and not accumulate),
                        stop=(dc == DC - 1),
                        perf_mode=mybir.MatmulPerfMode.DoubleRow,
                        skip_group_check=accumulate,
                    )

        # Three-stage software pipeline:
        #   phase_a(it): DMAs + q/k/v projections; per-n the PSUM k/v are
        #     drained to SBUF bf16 by ACT, then one DVE multiply + short
        #     tree produces the logits -- PSUM recycles at matmul pace.
        #   attn_chain(it-1): softmax + attn*v batched DVE ops.  Runs one
        #     tile behind so its serial tail never delays the per-n PSUM
        #     consumers of the CURRENT tile (in-order DVE!).
        #   phase_b(it-2): transpose + output projection + residual, two
        #     tiles behind so aout is long since ready.
        state_a = [None] * n_tiles
        state_c = [None] * n_tiles

        def phase_a(it):
            tok0 = it * P
            xt = inpool.tile([P, DC, 2, P], FP8, tag="xt", name="xt")
            nc.sync.dma_start(xt[:], curP[it])
            ht = inpool.tile([P, NPREV, DC, 2, P], FP8, tag="ht", name="ht")
            nc.sync.dma_start(ht[:], histP[it].rearrange("n p dc two t -> p n dc two t"))
            xr = xrpool.tile([P, D], F32, tag="xr", name="xr")
            nc.sync.dma_start(xr[:], xres[tok0 : tok0 + P, :])

            # phase_b for tile it-2 interleaves into this tile's PE
            # stream: transposes first (aout is long ready, and ACT then
            # queues the aoutT drains ahead of the q drain so the o-proj
            # at n==2 never waits), o-proj after two kv slots.
            if it >= 2 and not mm_only:
                phase_b_tp(it - 2)

            # q projection, then PSUM -> SBUF bf16 on the ACT engine
            qp = psum.tile([P, D], F32, tag="mm", name="qp")
            proj_dr(qp, lambda dc: xt[:, dc, :, :], "wq")
            q_sb = qpool.tile([P, D], BF16, tag="q", name="q_sb")
            nc.scalar.copy(q_sb[:], qp[:])

            # k,v projections per history slot; both drain to SBUF bf16
            # on ACT with DENSE writes (strided ACT writes measured 3.9x
            # slower on HW).  ACT is the only per-slot PSUM consumer, so
            # PSUM recycles at ACT pace and the DVE is off that path.
            v_sb = vpool.tile([P, NPREV, D], BF16, tag="v", name="v_sb")
            k_sb = kpool.tile([P, NPREV, D], BF16, tag="k", name="k_sb")
            for n in range(NPREV):
                if n == 2 and it >= 2 and not mm_only:
                    phase_b_oproj(it - 2)
                kp = psum.tile([P, D], F32, tag="mm", name="kp")
                vp = psum.tile([P, D], F32, tag="mm", name="vp")
                for out_psum, w_name in ((kp, "wk"), (vp, "wv")):
                    proj_dr(out_psum, lambda dc: ht[:, n, dc, :, :], w_name)
                if mm_only:
                    continue
                nc.scalar.copy(k_sb[:, n, :], kp[:])
                nc.scalar.copy(v_sb[:, n, :], vp[:])
            state_a[it] = (v_sb, k_sb, q_sb, xr, tok0)

        def attn_chain(it):
            v_sb, k_sb, q_sb, xr, tok0 = state_a[it]
            state_a[it] = None
            # Batched logits for all 8 slots in 4 DVE ops (bf16 SBUF
            # operands, innermost stride 1 -> 2x mode on the TensorTensors).
            prod = ppool.tile([P, NPREV, H, HD], BF16, tag="prod", name="prod")
            q_bc = bass.AP(
                q_sb[:].tensor, q_sb[:].offset, [q_sb[:].ap[0], [0, NPREV], [1, D]]
            )
            nc.vector.tensor_mul(
                prod.rearrange("p n h e -> p n (h e)"), q_bc,
                k_sb.rearrange("p n d -> p n d"),
            )
            # per-head sum over e=64: two stride-1 tree levels at 2x,
            # then one short strided-read reduce (TensorReduce is 1x-only;
            # all WRITES stay dense -- strided writes are slow on HW).
            logits = spool.tile([P, NPREV, H], BF16, tag="logits", name="logits")
            nc.vector.tensor_add(
                prod[:, :, :, 0:32], prod[:, :, :, 0:32], prod[:, :, :, 32:64]
            )
            nc.vector.tensor_add(
                prod[:, :, :, 0:16], prod[:, :, :, 0:16], prod[:, :, :, 16:32]
            )
            nc.vector.reduce_sum(
                logits[:], prod[:, :, :, 0:16], axis=mybir.AxisListType.X
            )
            # expt = exp(SCALE * logits)  (one batched ACT op)
            expt = spool.tile([P, NPREV, H], BF16, tag="expt", name="expt")
            nc.scalar.activation(
                expt.rearrange("p n h -> p (n h)"),
                logits.rearrange("p n h -> p (n h)"),
                mybir.ActivationFunctionType.Exp,
                scale=SCALE,
            )
            # softmax denominator over n (strided read), reciprocal
            ssum = spool.tile([P, H], F32, tag="ssum", name="ssum")
            nc.vector.reduce_sum(
                ssum[:],
                expt.rearrange("p n h -> p h n"),
                axis=mybir.AxisListType.X,
            )
            rsum = spool.tile([P, H], F32, tag="rsum", name="rsum")
            nc.vector.reciprocal(rsum[:], ssum[:])
            # attn[t, n, h] = expt * rsum (broadcast over n; tiny, 1x ok)
            attn = spool.tile([P, NPREV, H], BF16, tag="attn", name="attn")
            r_ap = rsum[:]
            r_bc = bass.AP(
                r_ap.tensor, r_ap.offset, [r_ap.ap[0], [0, NPREV], r_ap.ap[1]]
            )
            nc.vector.tensor_mul(attn[:], expt[:], r_bc)
            # v_sb[t, n, (h e)] *= attn[t, n, h] (broadcast over e, 1x:
            # no innermost-contiguous shape exists for both operands).
            # Slots 0-4 on DVE, 5-7 on the otherwise-idle Pool engine
            # (Pool is ~2x slower per element; split balances the time).
            a_ap = attn[:]
            v_view = v_sb.rearrange("p n (h e) -> p n h e", e=HD)
            for eng, n0, n1 in ((nc.vector, 0, 5), (nc.gpsimd, 5, 8)):
                a_bc = bass.AP(
                    a_ap.tensor,
                    a_ap.offset + n0 * H,
                    [a_ap.ap[0], [H, n1 - n0], [1, H], [0, HD]],
                )
                eng.tensor_mul(v_view[:, n0:n1], v_view[:, n0:n1], a_bc)
            # aout[t, d] = sum_n v_sb[t, n, d]: dense tree adds (2x mode)
            nc.vector.tensor_add(
                v_sb[:, 0:4, :], v_sb[:, 0:4, :], v_sb[:, 4:8, :]
            )
            nc.vector.tensor_add(
                v_sb[:, 0:2, :], v_sb[:, 0:2, :], v_sb[:, 2:4, :]
            )
            aout = aopool.tile([P, D], BF16, tag="aout", name="aout")
            nc.vector.tensor_add(aout[:], v_sb[:, 0, :], v_sb[:, 1, :])
            state_c[it] = (aout, xr, tok0)

        bstate = {}

        def phase_b_tp(it):
            aout, xr, tok0 = state_c[it]
            state_c[it] = None
            # transpose attention output 128x128 on the tensor engine;
            # ACT drains the bf16 psum to fp8 (DoubleRow o-proj stationary)
            aoutT = atpool.tile([P, C, P], FP8, tag="aoutT", name="aoutT")
            for g2 in range(2):
                tp = psum.tile([P, 4 * P], BF16, tag="mm", name="tp")
                for cc in range(4):
                    c = g2 * 4 + cc
                    nc.tensor.transpose(
                        tp[:, cc * P : (cc + 1) * P],
                        aout[:, c * P : (c + 1) * P],
                        ident[:],
                    )
                nc.scalar.copy(
                    aoutT[:, g2 * 4 : (g2 + 1) * 4, :].rearrange("p c t -> p (c t)"),
                    tp[:],
                )
            bstate[it] = (aoutT, xr, tok0)

        def phase_b_oproj(it):
            aoutT, xr, tok0 = bstate.pop(it)
            # output projection (fp8 DoubleRow) + fp32 residual add (DVE)
            yp = psum.tile([P, D], F32, tag="mm", name="yp")
            aview = aoutT.rearrange("p (dc two) t -> p dc two t", two=2)
            proj_dr(yp, lambda dc: aview[:, dc, :, :], "wo")
            y_sb = ypool.tile([P, D], F32, tag="ysb", name="y_sb")
            nc.vector.tensor_add(y_sb[:], yp[:], xr[:])
            nc.sync.dma_start(y[tok0 : tok0 + P, :], y_sb[:])

        def phase_b(it):
            phase_b_tp(it)
            phase_b_oproj(it)

        def whole_body():
            if mm_only:
                for it in range(n_tiles):
                    phase_a(it)
                    v_sb, logits, xr, tok0 = state_a[it]
                    state_a[it] = None
                    y_sb = ypool.tile([P, D], F32, tag="ysb", name="y_sb")
                    nc.vector.tensor_copy(y_sb[:], xr[:])
                    nc.sync.dma_start(y[tok0 : tok0 + P, :], y_sb[:])
                return
            for it in range(n_tiles + 2):
                # attn_chain(it-1) first: its DVE ops' deps are all ready,
                # so the in-order DVE queue never idles waiting on tile
                # it's ACT drains while older work is available.
                if 1 <= it <= n_tiles:
                    attn_chain(it - 1)
                if it < n_tiles:
                    phase_a(it)  # phase_b(it-2) is interleaved inside
                if it >= n_tiles:
                    phase_b(it - 2)  # pipeline drain

        if repeat == 1:
            whole_body()
        else:
            assert repeat % UNROLL == 0, repeat
            with tc.For_i(0, repeat // UNROLL, 1):
                for _ in range(UNROLL):
                    whole_body()

    nc.compile()
    return nc


def prep_inputs(current, history, Wq, Wk, Wv, Wo, n_cores=N_CORES):
    """Host-side shard + layout prep.  Returns per-core input maps."""
    f8 = ml_dtypes.float8_e4m3
    cur = np.ascontiguousarray(current.reshape(BS, D)).astype(np.float32)
    hist = history.reshape(BS, NPREV, D)

    n_tiles_total = BS // P
    # curP[tile, p, dc, two, t] = cur[tile*128 + t, (2dc+two)*128 + p]
    cur_f8 = cur.astype(f8)
    curP = np.ascontiguousarray(
        cur_f8.reshape(n_tiles_total, P, DC, 2, P).transpose(0, 4, 2, 3, 1)
    )
    # histP[tile, n, p, dc, two, t] = hist[tile*128 + t, n, (2dc+two)*128 + p]
    hist_f8 = hist.astype(f8)
    histP = np.ascontiguousarray(
        hist_f8.reshape(n_tiles_total, P, NPREV, DC, 2, P).transpose(0, 2, 5, 3, 4, 1)
    )

    def wprep8(w):
        # w?T8[dc, p, two, dout] = W.T[(2dc+two)*128+p, dout] (fp8)
        return np.ascontiguousarray(
            w.T.astype(f8).reshape(DC, 2, P, D).transpose(0, 2, 1, 3)
        )

    wqT, wkT, wvT, woT = (wprep8(w) for w in (Wq, Wk, Wv, Wo))

    tiles_per_core = n_tiles_total // n_cores
    in_maps = []
    for ci in range(n_cores):
        t0 = ci * tiles_per_core
        sl = slice(ci * T_CORE, (ci + 1) * T_CORE)
        in_maps.append(
            {
                "curP": np.ascontiguousarray(curP[t0 : t0 + tiles_per_core]),
                "histP": np.ascontiguousarray(histP[t0 : t0 + tiles_per_core]),
                "xres": np.ascontiguousarray(cur[sl]),
                "wqT": wqT,
                "wkT": wkT,
                "wvT": wvT,
                "woT": woT,
            }
        )
    return in_maps


def kernel(current, history, Wq, Wk, Wv, Wo):
    from concourse.bass_utils import run_bass_kernel_spmd

    if "nc" not in _CACHE:
        _CACHE["nc"] = build_program(N_TILES)
    nc = _CACHE["nc"]

    in_maps = prep_inputs(current, history, Wq, Wk, Wv, Wo)
    results = run_bass_kernel_spmd(nc, in_maps, core_ids=list(range(N_CORES))).results
    y = np.concatenate([results[ci]["y"] for ci in range(N_CORES)], axis=0)
    return y.reshape(B, S, D).astype(np.float32)


# revision 13
# speedup vs baseline: 1.0672x; 1.0672x over previous
"""CrossDepthAttention Trainium2 kernel.

Computation (per token t):
    q = x_t @ Wq.T
    k_n = h_{t,n} @ Wk.T, v_n = h_{t,n} @ Wv.T   for n in 0..7
    logits[h,n] = <q_h, k_{n,h}> / sqrt(Hd)
    attn = softmax_n(logits)
    out_h = sum_n attn[h,n] * v_{n,h}
    y_t = x_t + out @ Wo.T

Sharding: pure data-parallel over the B*S = 8192 tokens, 1024 tokens per
core on 8 cores (every token attends only to its own history: no
cross-token coupling, no collective).

Per-core kernel, token tiles of 128.  All 18 projections run on the
tensor engine in fp8e4m3 with the DoubleRow perf mode: stationary
[128, 2, 128] / moving [128, 2, 512] pairs contract 256 deep per output
column.  Measured 1.03 cyc/out-col with the stationary reused across the
two 512-wide psum halves (dc outer, j inner) -- a 2.05x throughput win
over bf16; the doubled ldweights hides completely only when the
stationary is reused by back-to-back matmuls.  End-to-end rel err with
everything fp8 is 1.2e-2 vs the 2e-2 budget (numpy-emulated and
HW-confirmed; fp32 psum accumulation throughout).

Attention runs on DVE/ACT in bf16 with every big TensorTensor shaped for
the DVE 2x_1p mode (all operands 2-byte, innermost stride 1):
  * k and v drain PSUM->SBUF bf16 on the ACT engine (so PSUM recycles at
    matmul pace and the q*k product reads 2-byte SBUF, not 4-byte PSUM).
  * v is stored [t, d, n] so the attn broadcast multiply and the
    n-reduction tree are innermost-contiguous (stride-1 over n).
  * logits = per-head sum of q*k: two 2x tree levels then one short
    strided reduce (TensorReduce has no fast mode, so shrink its input).
  * the fp32 residual add runs on the otherwise-idle Pool engine.
"""

import numpy as np
import ml_dtypes
from contextlib import ExitStack

import concourse.bass as bass
import concourse.mybir as mybir
import concourse.tile as tile
from concourse import bacc
from concourse.masks import make_identity

BF16 = mybir.dt.bfloat16
FP8 = mybir.dt.float8e4
F32 = mybir.dt.float32

B, S, D = 4, 2048, 1024
NPREV = 8
H = 16
HD = D // H  # 64
BS = B * S
N_CORES = 8
T_CORE = BS // N_CORES  # 1024 tokens per core
P = 128  # partition / token-tile size
C = D // P  # 8 contraction chunks; DC = 4 DoubleRow double-chunks
DC = C // 2
N_TILES = T_CORE // P  # 8 token tiles per core
HALF = 512  # matmul free-dim (one PSUM bank)
SCALE = 1.0 / float(np.sqrt(HD))

_CACHE = {}


UNROLL = 4


def build_program(n_tiles=N_TILES, repeat=1, mm_only=False):
    """Build the single-core Bass/Tile program (run SPMD on 8 cores).

    repeat>1 wraps the whole computation in an on-device For_i loop of
    repeat//UNROLL iterations with UNROLL copies of the body each --
    used only for timing (amortizes host dispatch overhead away, and the
    partial unroll amortizes For_i's per-iteration all-engine barrier,
    letting consecutive kernel executions overlap through pool WAR deps).
    """
    nc = bacc.Bacc("TRN2", debug=False, num_devices=N_CORES)
    t_tok = n_tiles * P

    curP = nc.dram_tensor("curP", [n_tiles, P, DC, 2, P], FP8, kind="ExternalInput").ap()
    histP = nc.dram_tensor(
        "histP", [n_tiles, NPREV, P, DC, 2, P], FP8, kind="ExternalInput"
    ).ap()
    xres = nc.dram_tensor("xres", [t_tok, D], F32, kind="ExternalInput").ap()
    wq = nc.dram_tensor("wqT", [DC, P, 2, D], FP8, kind="ExternalInput").ap()
    wk = nc.dram_tensor("wkT", [DC, P, 2, D], FP8, kind="ExternalInput").ap()
    wv = nc.dram_tensor("wvT", [DC, P, 2, D], FP8, kind="ExternalInput").ap()
    wo = nc.dram_tensor("woT", [DC, P, 2, D], FP8, kind="ExternalInput").ap()
    y = nc.dram_tensor("y", [t_tok, D], F32, kind="ExternalOutput").ap()

    with ExitStack() as ctx:
        ctx.enter_context(
            nc.allow_low_precision(reason="fp8/bf16 attn validated vs 2e-2")
        )
        tc = ctx.enter_context(tile.TileContext(nc))

        wpool = ctx.enter_context(tc.tile_pool(name="wpool", bufs=1))
        inpool = ctx.enter_context(tc.tile_pool(name="inpool", bufs=2))
        xrpool = ctx.enter_context(tc.tile_pool(name="xrpool", bufs=3))
        qpool = ctx.enter_context(tc.tile_pool(name="qpool", bufs=2))
        kpool = ctx.enter_context(tc.tile_pool(name="kpool", bufs=2))
        vpool = ctx.enter_context(tc.tile_pool(name="vpool", bufs=2))
        ppool = ctx.enter_context(tc.tile_pool(name="ppool", bufs=2))
        spool = ctx.enter_context(tc.tile_pool(name="spool", bufs=2))
        aopool = ctx.enter_context(tc.tile_pool(name="aopool", bufs=2))
        atpool = ctx.enter_context(tc.tile_pool(name="atpool", bufs=2))
        ypool = ctx.enter_context(tc.tile_pool(name="ypool", bufs=2))
        psum = ctx.enter_context(tc.tile_pool(name="psum", bufs=4, space="PSUM"))

        # Resident weights, DoubleRow moving layout:
        # w8_sb[p, dc, two, dout] = W?T[(2dc+two)*128+p, dout]
        w_sb = {}
        for name, dram in (("wq", wq), ("wk", wk), ("wv", wv), ("wo", wo)):
            wt = wpool.tile([P, DC, 2, D], FP8, name=f"{name}_sb", tag=name)
            nc.sync.dma_start(wt[:], dram.rearrange("dc p two d -> p dc two d"))
            w_sb[name] = wt
        ident = wpool.tile([P, P], BF16, name="ident", tag="ident")
        make_identity(nc, ident[:])

        def proj_dr(out_psum, stat, w_name, accumulate=False):
            """fp8 DoubleRow projection: out[t,:] (+)= stat.T @ W.T with
            256-deep contraction per column.  dc outer / j inner so each
            stationary serves two back-to-back matmuls -- that reuse is
            what hides the doubled ldweights (measured 1.03 cyc/col vs
            1.26 with j outer)."""
            wt = w_sb[w_name]
            for dc in range(DC):
                for j in range(2):
                    nc.tensor.matmul(
                        out_psum[:, j * HALF : (j + 1) * HALF],
                        lhsT=stat(dc),
                        rhs=wt[:, dc, :, j * HALF : (j + 1) * HALF],
                        start=(dc == 0 and not accumulate),
                        stop=(dc == DC - 1),
                        perf_mode=mybir.MatmulPerfMode.DoubleRow,
                        skip_group_check=accumulate,
                    )

        # Three-stage software pipeline:
        #   phase_a(it): DMAs + q/k/v projections; per-n the PSUM k/v are
        #     drained to SBUF bf16 by ACT, then one DVE multiply + short
        #     tree produces the logits -- PSUM recycles at matmul pace.
        #   attn_chain(it-1): softmax + attn*v batched DVE ops.  Runs one
        #     tile behind so its serial tail never delays the per-n PSUM
        #     consumers of the CURRENT tile (in-order DVE!).
        #   phase_b(it-2): transpose + output projection + residual, two
        #     tiles behind so aout is long since ready.
        state_a = {}
        state_c = {}

        def phase_a(g):
            it = g % n_tiles
            tok0 = it * P
            xt = inpool.tile([P, DC, 2, P], FP8, tag="xt", name="xt")
            nc.sync.dma_start(xt[:], curP[it])
            ht = inpool.tile([P, NPREV, DC, 2, P], FP8, tag="ht", name="ht")
            nc.sync.dma_start(ht[:], histP[it].rearrange("n p dc two t -> p n dc two t"))
            xr = xrpool.tile([P, D], F32, tag="xr", name="xr")
            nc.sync.dma_start(xr[:], xres[tok0 : tok0 + P, :])

            # q projection, then PSUM -> SBUF bf16 on the ACT engine
            qp = psum.tile([P, D], F32, tag="mm", name="qp")
            proj_dr(qp, lambda dc: xt[:, dc, :, :], "wq")
            q_sb = qpool.tile([P, D], BF16, tag="q", name="q_sb")
            nc.scalar.copy(q_sb[:], qp[:])

            # phase_b for tile it-2 interleaves into this tile's PE
            # stream: transposes now, o-proj after two kv slots, so the
            # ACT psum->sbuf copy latencies hide under kv matmuls and
            # the PE never idles at the tile tail.
            if g >= 2 and not mm_only:
                phase_b_tp(g - 2)

            # k,v projections per history slot; both drain to SBUF bf16
            # on ACT with DENSE writes (strided ACT writes measured 3.9x
            # slower on HW).  ACT is the only per-slot PSUM consumer, so
            # PSUM recycles at ACT pace and the DVE is off that path.
            v_sb = vpool.tile([P, NPREV, D], BF16, tag="v", name="v_sb")
            k_sb = kpool.tile([P, NPREV, D], BF16, tag="k", name="k_sb")
            for n in range(NPREV):
                if n == 2 and g >= 2 and not mm_only:
                    phase_b_oproj(g - 2)
                kp = psum.tile([P, D], F32, tag="mm", name="kp")
                vp = psum.tile([P, D], F32, tag="mm", name="vp")
                for out_psum, w_name in ((kp, "wk"), (vp, "wv")):
                    proj_dr(out_psum, lambda dc: ht[:, n, dc, :, :], w_name)
                if mm_only:
                    continue
                nc.scalar.copy(k_sb[:, n, :], kp[:])
                nc.scalar.copy(v_sb[:, n, :], vp[:])
            state_a[g] = (v_sb, k_sb, q_sb, xr, tok0)

        def attn_chain(g):
            v_sb, k_sb, q_sb, xr, tok0 = state_a.pop(g)
            # Batched logits for all 8 slots in 4 DVE ops (bf16 SBUF
            # operands, innermost stride 1 -> 2x mode on the TensorTensors).
            prod = ppool.tile([P, NPREV, H, HD], BF16, tag="prod", name="prod")
            q_bc = bass.AP(
                q_sb[:].tensor, q_sb[:].offset, [q_sb[:].ap[0], [0, NPREV], [1, D]]
            )
            nc.vector.tensor_mul(
                prod.rearrange("p n h e -> p n (h e)"), q_bc,
                k_sb.rearrange("p n d -> p n d"),
            )
            # per-head sum over e=64: two stride-1 tree levels at 2x,
            # then one short strided-read reduce (TensorReduce is 1x-only;
            # all WRITES stay dense -- strided writes are slow on HW).
            logits = spool.tile([P, NPREV, H], BF16, tag="logits", name="logits")
            nc.vector.tensor_add(
                prod[:, :, :, 0:32], prod[:, :, :, 0:32], prod[:, :, :, 32:64]
            )
            nc.vector.tensor_add(
                prod[:, :, :, 0:16], prod[:, :, :, 0:16], prod[:, :, :, 16:32]
            )
            nc.vector.reduce_sum(
                logits[:], prod[:, :, :, 0:16], axis=mybir.AxisListType.X
            )
            # expt = exp(SCALE * logits)  (one batched ACT op)
            expt = spool.tile([P, NPREV, H], BF16, tag="expt", name="expt")
            nc.scalar.activation(
                expt.rearrange("p n h -> p (n h)"),
                logits.rearrange("p n h -> p (n h)"),
                mybir.ActivationFunctionType.Exp,
                scale=SCALE,
            )
            # softmax denominator over n (strided read), reciprocal
            ssum = spool.tile([P, H], F32, tag="ssum", name="ssum")
            nc.vector.reduce_sum(
                ssum[:],
                expt.rearrange("p n h -> p h n"),
                axis=mybir.AxisListType.X,
            )
            rsum = spool.tile([P, H], F32, tag="rsum", name="rsum")
            nc.vector.reciprocal(rsum[:], ssum[:])
            # attn[t, n, h] = expt * rsum (broadcast over n; tiny, 1x ok)
            attn = spool.tile([P, NPREV, H], BF16, tag="attn", name="attn")
            r_ap = rsum[:]
            r_bc = bass.AP(
                r_ap.tensor, r_ap.offset, [r_ap.ap[0], [0, NPREV], r_ap.ap[1]]
            )
            nc.vector.tensor_mul(attn[:], expt[:], r_bc)
            # v_sb[t, n, (h e)] *= attn[t, n, h] (broadcast over e, 1x:
            # no innermost-contiguous shape exists for both operands).
            # Slots 0-4 on DVE, 5-7 on the otherwise-idle Pool engine
            # (Pool is ~2x slower per element; split balances the time).
            a_ap = attn[:]
            v_view = v_sb.rearrange("p n (h e) -> p n h e", e=HD)
            for eng, n0, n1 in ((nc.vector, 0, 5), (nc.gpsimd, 5, 8)):
                a_bc = bass.AP(
                    a_ap.tensor,
                    a_ap.offset + n0 * H,
                    [a_ap.ap[0], [H, n1 - n0], [1, H], [0, HD]],
                )
                eng.tensor_mul(v_view[:, n0:n1], v_view[:, n0:n1], a_bc)
            # aout[t, d] = sum_n v_sb[t, n, d]: dense tree adds (2x mode)
            nc.vector.tensor_add(
                v_sb[:, 0:4, :], v_sb[:, 0:4, :], v_sb[:, 4:8, :]
            )
            nc.vector.tensor_add(
                v_sb[:, 0:2, :], v_sb[:, 0:2, :], v_sb[:, 2:4, :]
            )
            aout = aopool.tile([P, D], BF16, tag="aout", name="aout")
            nc.vector.tensor_add(aout[:], v_sb[:, 0, :], v_sb[:, 1, :])
            state_c[g] = (aout, xr, tok0)

        bstate = {}

        def phase_b_tp(g):
            aout, xr, tok0 = state_c.pop(g)
            # transpose attention output 128x128 on the tensor engine;
            # ACT drains the bf16 psum to fp8 (DoubleRow o-proj stationary)
            aoutT = atpool.tile([P, C, P], FP8, tag="aoutT", name="aoutT")
            for g2 in range(2):
                tp = psum.tile([P, 4 * P], BF16, tag="mm", name="tp")
                for cc in range(4):
                    c = g2 * 4 + cc
                    nc.tensor.transpose(
                        tp[:, cc * P : (cc + 1) * P],
                        aout[:, c * P : (c + 1) * P],
                        ident[:],
                    )
                nc.scalar.copy(
                    aoutT[:, g2 * 4 : (g2 + 1) * 4, :].rearrange("p c t -> p (c t)"),
                    tp[:],
                )
            bstate[g] = (aoutT, xr, tok0)

        def phase_b_oproj(g):
            aoutT, xr, tok0 = bstate.pop(g)
            # output projection (fp8 DoubleRow) + fp32 residual add (DVE)
            yp = psum.tile([P, D], F32, tag="mm", name="yp")
            aview = aoutT.rearrange("p (dc two) t -> p dc two t", two=2)
            proj_dr(yp, lambda dc: aview[:, dc, :, :], "wo")
            y_sb = ypool.tile([P, D], F32, tag="ysb", name="y_sb")
            nc.vector.tensor_add(y_sb[:], yp[:], xr[:])
            nc.sync.dma_start(y[tok0 : tok0 + P, :], y_sb[:])

        def phase_b(g):
            phase_b_tp(g)
            phase_b_oproj(g)

        def whole_body(units=1):
            if mm_only:
                for it in range(n_tiles):
                    phase_a(it)
                    v_sb, k_sb, q_sb, xr, tok0 = state_a.pop(it)
                    y_sb = ypool.tile([P, D], F32, tag="ysb", name="y_sb")
                    nc.vector.tensor_copy(y_sb[:], xr[:])
                    nc.sync.dma_start(y[tok0 : tok0 + P, :], y_sb[:])
                return
            # units>1: consecutive kernel executions are software-pipelined
            # through a GLOBAL tile counter g -- the drain work of unit u's
            # last two tiles interleaves into unit u+1's projection stream,
            # so the PE only stalls at the one For_i barrier per lap.
            last = units * n_tiles - 1
            for g in range(units * n_tiles):
                # attn_chain(g-1) first: its DVE ops' deps are all ready,
                # so the in-order DVE queue never idles waiting on tile
                # g's ACT drains while older work is available.
                if g >= 1:
                    attn_chain(g - 1)
                phase_a(g)  # phase_b(g-2) is interleaved inside
            attn_chain(last)
            phase_b(last - 1)
            phase_b(last)

        if repeat == 1:
            whole_body()
        else:
            assert repeat % UNROLL == 0, repeat
            with tc.For_i(0, repeat // UNROLL, 1):
                whole_body(UNROLL)

    nc.compile()
    return nc


def prep_inputs(current, history, Wq, Wk, Wv, Wo, n_cores=N_CORES):
    """Host-side shard + layout prep.  Returns per-core input maps."""
    f8 = ml_dtypes.float8_e4m3
    cur = np.ascontiguousarray(current.reshape(BS, D)).astype(np.float32)
    hist = history.reshape(BS, NPREV, D)

    n_tiles_total = BS // P
    # curP[tile, p, dc, two, t] = cur[tile*128 + t, (2dc+two)*128 + p]
    cur_f8 = cur.astype(f8)
    curP = np.ascontiguousarray(
        cur_f8.reshape(n_tiles_total, P, DC, 2, P).transpose(0, 4, 2, 3, 1)
    )
    # histP[tile, n, p, dc, two, t] = hist[tile*128 + t, n, (2dc+two)*128 + p]
    hist_f8 = hist.astype(f8)
    histP = np.ascontiguousarray(
        hist_f8.reshape(n_tiles_total, P, NPREV, DC, 2, P).transpose(0, 2, 5, 3, 4, 1)
    )

    def wprep8(w):
        # w?T8[dc, p, two, dout] = W.T[(2dc+two)*128+p, dout] (fp8)
        return np.ascontiguousarray(
            w.T.astype(f8).reshape(DC, 2, P, D).transpose(0, 2, 1, 3)
        )

    wqT, wkT, wvT, woT = (wprep8(w) for w in (Wq, Wk, Wv, Wo))

    tiles_per_core = n_tiles_total // n_cores
    in_maps = []
    for ci in range(n_cores):
        t0 = ci * tiles_per_core
        sl = slice(ci * T_CORE, (ci + 1) * T_CORE)
        in_maps.append(
            {
                "curP": np.ascontiguousarray(curP[t0 : t0 + tiles_per_core]),
                "histP": np.ascontiguousarray(histP[t0 : t0 + tiles_per_core]),
                "xres": np.ascontiguousarray(cur[sl]),
                "wqT": wqT,
                "wkT": wkT,
                "wvT": wvT,
                "woT": woT,
            }
        )
    return in_maps


def kernel(current, history, Wq, Wk, Wv, Wo):
    from concourse.bass_utils import run_bass_kernel_spmd

    if "nc" not in _CACHE:
        _CACHE["nc"] = build_program(N_TILES)
    nc = _CACHE["nc"]

    in_maps = prep_inputs(current, history, Wq, Wk, Wv, Wo)
    results = run_bass_kernel_spmd(nc, in_maps, core_ids=list(range(N_CORES))).results
    y = np.concatenate([results[ci]["y"] for ci in range(N_CORES)], axis=0)
    return y.reshape(B, S, D).astype(np.float32)
